# revision 1
# baseline (speedup 1.0000x reference)
"""Multi-head cross-attention Trainium2 kernel (8 NeuronCores, SPMD).

Problem: tokens [4, 4096, 1024], context [4, 1024, 768], 16 heads, d=64.
  Q = tokens @ Wq; K = context @ Wk; V = context @ Wv
  out = softmax(Q K^T / 8) V  -> @ Wo + bo

Sharding: 8 cores = (batch b in 0..3) x (query-row half in 0..1).
Each core handles 2048 query rows of one batch against that batch's full
1024-key context.  No collectives; host concatenates the 8 output chunks.

Per-core dataflow (fp32 PSUM accumulation everywhere; projections contract
bf16 operands; score and output matmuls use float32r storage, which runs at
bf16 speed for moving dims >= 256 and carries ~13-bit mantissas):
  - host passes tokens-chunk TRANSPOSED (at = [1024, 2048]) and context
    transposed (ct = [768, 1024]) so no on-device transposes are needed.
  - K^T proj:  kt[fc] = (Wk[:, fc]).T-contract ct   -> [128 feats, 1024 keys]
    stored f32r (feature chunk fc = head pair 2fc, 2fc+1 on partitions
    0:64 / 64:128)
  - V proj:    vp[kc] = [keys 128, 16*(64+1)] bf16 with a ones-column per
    head (the ones-column makes P@V' also produce the softmax denominators)
  - Q^T proj per row-block of 512 rows, stored f32r.
  - attention per (row-block, head-pair): S^T tiles [128 keys, 512 rows]
    via row-tiled PAIRED f32r matmuls (k=64 each, tile_position
    (0,0)/(64,0) concurrent); exp on ScalarE with scale=1/8 folded in;
    P@V' bf16 matmuls accumulate [65, 512] per head over 8 key chunks
    (row 64 = denominator).
  - denominators round-trip through DRAM to batch the reciprocal across
    heads ([16, 512] on 16 partitions, reciprocal_approx_fast) and
    partition-broadcast the reciprocals back (0-step DMA), then normalize
    O (f32r) in SBUF.
  - Y = O^T-contract Wo in f32r + bias, written out f32.
"""

import numpy as np
import ml_dtypes

B = 4
N = 4096
HID = 1024
CTX = 768
M = 1024          # context length (keys)
H = 16
D = 64
NCORES = 8
R = N * B // NCORES   # 2048 query rows per core
RB = 512              # row block
NRB = R // RB         # 4
FC = HID // 128       # 8 feature chunks == head pairs
KC = M // 128         # 8 key chunks
ICQ = HID // 128      # 8 contraction chunks for Q/Y proj
ICC = CTX // 128      # 6 contraction chunks for K/V proj
SCALE = D ** -0.5

_CACHE = {}


def _body(tc, ctx_stack, at, ct, wq, wk, wv, wo, bo, y):
    import concourse.bass as bass
    from concourse import mybir

    nc = tc.nc
    F32, BF16 = mybir.dt.float32, mybir.dt.bfloat16
    F32R = mybir.dt.float32r
    EXP = mybir.ActivationFunctionType.Exp
    MUL = mybir.AluOpType.mult
    ADD = mybir.AluOpType.add
    enter = ctx_stack.enter_context

    p_w = enter(tc.tile_pool(name="p_w", bufs=14))
    p_at = enter(tc.tile_pool(name="p_at", bufs=8))
    p_qt = enter(tc.tile_pool(name="p_qt", bufs=12))
    p_kt = enter(tc.tile_pool(name="p_kt", bufs=8))
    p_vp = enter(tc.tile_pool(name="p_vp", bufs=8))
    p_e = enter(tc.tile_pool(name="p_e", bufs=5))
    p_o = enter(tc.tile_pool(name="p_o", bufs=16))
    p_dn = enter(tc.tile_pool(name="p_dn", bufs=2))
    p_sm = enter(tc.tile_pool(name="p_sm", bufs=2))
    p_bc = enter(tc.tile_pool(name="p_bc", bufs=2))
    p_y = enter(tc.tile_pool(name="p_y", bufs=2))
    p_wo = enter(tc.tile_pool(name="p_wo", bufs=8))
    p_1 = enter(tc.tile_pool(name="p_1", bufs=1))
    ps_s = enter(tc.tile_pool(name="ps_s", bufs=2, space="PSUM"))
    ps_m = enter(tc.tile_pool(name="ps_m", bufs=2, space="PSUM"))
    ps_o = enter(tc.tile_pool(name="ps_o", bufs=2, space="PSUM"))
    p_dram = enter(tc.tile_pool(name="p_dram", bufs=1, space="DRAM"))

    # ---- PE warm-up: ~3.5us of dummy matmuls during the initial DMA
    # window flips the HAM clock gate to 2.4 GHz before real work ----
    warm_t = p_y.tile([128, HID], F32, name="warm_t", tag="y")
    nc.vector.memset(warm_t[:, 0:512], 0.0)
    for i in range(3):
        wps = ps_m.tile([128, 512], F32, name="wps", tag="m")
        nc.tensor.matmul(wps, warm_t[:, 0:128], warm_t[:, 0:512],
                         start=True, stop=True)
    # dummy exp pre-loads the ScalarE activation table set off the
    # attention critical path
    nc.scalar.activation(warm_t[:, 8:16], warm_t[:, 0:8], EXP, scale=SCALE)

    # ---- bias broadcast [1, HID] -> [128, HID] (gpsimd 0-step DMA) ----
    bias_sb = p_1.tile([128, HID], F32, name="bias_sb", tag="bias")
    nc.gpsimd.dma_start(
        out=bias_sb,
        in_=bass.AP(tensor=bo.tensor, offset=bo.offset,
                    ap=[[0, 128]] + [list(a) for a in bo.ap[1:]]),
    )

    # ---- load ct / wk / wv (shared slot tag "w") ----
    ct_sb = []
    for i in range(ICC):
        t = p_w.tile([128, M], BF16, name=f"ct{i}", tag="w")
        nc.sync.dma_start(out=t, in_=ct[i * 128:(i + 1) * 128, :])
        ct_sb.append(t)
    wk_sb = []
    for i in range(ICC):
        t = p_w.tile([128, HID], BF16, name=f"wk{i}", tag="w")
        nc.sync.dma_start(out=t, in_=wk[i * 128:(i + 1) * 128, :])
        wk_sb.append(t)
    wv_sb = []
    for i in range(ICC):
        t = p_w.tile([128, HID], BF16, name=f"wv{i}", tag="w")
        nc.sync.dma_start(out=t, in_=wv[i * 128:(i + 1) * 128, :])
        wv_sb.append(t)

    # ---- K^T projection: kt[fc] = [128 feats, M keys] ----
    kt_sb = []
    for fc in range(FC):
        kt = p_kt.tile([128, M], F32R, name=f"kt{fc}", tag="kt")
        for half in range(2):
            ps = ps_m.tile([128, 512], F32, name="psk", tag="m")
            for ic in range(ICC):
                nc.tensor.matmul(
                    ps,
                    wk_sb[ic][:, fc * 128:(fc + 1) * 128],
                    ct_sb[ic][:, half * 512:(half + 1) * 512],
                    start=(ic == 0), stop=(ic == ICC - 1),
                )
            nc.vector.tensor_copy(kt[:, half * 512:(half + 1) * 512], ps)
        kt_sb.append(kt)

    # ---- V projection into [keys 128, 16 heads x (64 vals + 1 one)] ----
    vp_sb = []
    for kc in range(KC):
        vp = p_vp.tile([128, H * (D + 1)], BF16, name=f"vp{kc}", tag="vp")
        vpv = vp.rearrange("p (h c) -> p h c", h=H)
        nc.vector.memset(vpv[:, :, D:D + 1], 1.0)
        for half in range(2):
            ps = ps_m.tile([128, 512], F32, name="psv", tag="m")
            for ic in range(ICC):
                nc.tensor.matmul(
                    ps,
                    ct_sb[ic][:, kc * 128:(kc + 1) * 128],
                    wv_sb[ic][:, half * 512:(half + 1) * 512],
                    start=(ic == 0), stop=(ic == ICC - 1),
                )
            nc.vector.tensor_copy(
                vpv[:, half * 8:(half + 1) * 8, 0:D],
                ps.rearrange("p (h c) -> p h c", h=8),
            )
        vp_sb.append(vp)

    # ---- load wq then wo (reuse "w" slots as ct/wk/wv die) ----
    wq_sb = []
    for i in range(ICQ):
        t = p_w.tile([128, HID], BF16, name=f"wq{i}", tag="w")
        nc.sync.dma_start(out=t, in_=wq[i * 128:(i + 1) * 128, :])
        wq_sb.append(t)
    wo_sb = []
    for i in range(FC):
        t = p_wo.tile([128, HID], F32R, name=f"wo{i}", tag="wo")
        nc.sync.dma_start(out=t, in_=wo[i * 128:(i + 1) * 128, :].bitcast(F32R))
        wo_sb.append(t)

    dn_dram = p_dram.tile([NRB, H, RB], F32, name="dn_dram", tag="dn_dram")
    rc_dram = p_dram.tile([NRB, H, RB], F32, name="rc_dram", tag="rc_dram")

    def qload(rb):
        ats = []
        for ic in range(ICQ):
            a = p_at.tile([128, RB], BF16, name=f"at{ic}_{rb}", tag="at")
            nc.sync.dma_start(
                out=a, in_=at[ic * 128:(ic + 1) * 128, rb * RB:(rb + 1) * RB])
            ats.append(a)
        return ats

    def qproj_fc(rb, fc, ats, qts):
        ps = ps_m.tile([128, RB], F32, name="psq", tag="m")
        for ic in range(ICQ):
            nc.tensor.matmul(
                ps, wq_sb[ic][:, fc * 128:(fc + 1) * 128], ats[ic],
                start=(ic == 0), stop=(ic == ICQ - 1))
        qt = p_qt.tile([128, RB], F32R, name=f"qt{fc}_{rb}", tag="qt")
        nc.vector.tensor_copy(qt, ps)
        qts.append(qt)

    def qproj(rb):
        ats = qload(rb)
        qts = []
        for fc in range(FC):
            qproj_fc(rb, fc, ats, qts)
        return qts

    def attn(rb, hp, qts):
        h1, h2 = 2 * hp, 2 * hp + 1
        psO1 = ps_o.tile([128, RB], F32, name="psO1", tag="o")
        psO2 = ps_o.tile([128, RB], F32, name="psO2", tag="o")
        for kc in range(KC):
            psS = ps_s.tile([128, 2 * RB], F32, name="psS", tag="s")
            # paired row-tiled S^T matmuls: k=64 at partitions 0:64 / 64:128
            with tc.high_priority(offset=100):
                nc.tensor.matmul(
                    psS[:, 0:RB],
                    kt_sb[hp][0:64, kc * 128:(kc + 1) * 128],
                    qts[hp][0:64, :], start=True, stop=True)
                nc.tensor.matmul(
                    psS[:, RB:2 * RB],
                    kt_sb[hp][64:128, kc * 128:(kc + 1) * 128],
                    qts[hp][64:128, :], start=True, stop=True)
            e = p_e.tile([128, 2 * RB], BF16, name="e", tag="e")
            nc.scalar.activation(e, psS, EXP, scale=SCALE)
            nc.tensor.matmul(
                psO1[0:D + 1, :],
                vp_sb[kc][:, h1 * (D + 1):(h1 + 1) * (D + 1)],
                e[:, 0:RB], start=(kc == 0), stop=(kc == KC - 1))
            nc.tensor.matmul(
                psO2[0:D + 1, :],
                vp_sb[kc][:, h2 * (D + 1):(h2 + 1) * (D + 1)],
                e[:, RB:2 * RB], start=(kc == 0), stop=(kc == KC - 1))
        o = p_o.tile([128, RB], F32R, name=f"o{hp}_{rb}", tag="o")
        nc.vector.tensor_copy(o[0:64, :], psO1[0:D, :])
        nc.vector.tensor_copy(o[64:128, :], psO2[0:D, :])
        d1 = p_dn.tile([1, RB], F32, name="d1", tag="dn")
        nc.vector.tensor_copy(d1, psO1[D:D + 1, :])
        nc.sync.dma_start(out=dn_dram[rb, h1], in_=d1)
        d2 = p_dn.tile([1, RB], F32, name="d2", tag="dn")
        nc.vector.tensor_copy(d2, psO2[D:D + 1, :])
        nc.sync.dma_start(out=dn_dram[rb, h2], in_=d2)
        return o

    def norm_half(rb, q, o_tiles):
        h0 = q * (H // 4)
        dn_sb = p_sm.tile([H // 4, RB], F32, name="dn_sb", tag="sm")
        nc.sync.dma_start(out=dn_sb, in_=dn_dram[rb, h0:h0 + H // 4, :])
        rc_sb = p_sm.tile([H // 4, RB], F32, name="rc_sb", tag="sm")
        nc.vector.reciprocal_approx_fast(out=rc_sb, in_=dn_sb)
        nc.sync.dma_start(out=rc_dram[rb, h0:h0 + H // 4, :], in_=rc_sb)
        for hp in range(q * (FC // 4), (q + 1) * (FC // 4)):
            bc = p_bc.tile([128, RB], F32, name="bc", tag="bc")
            src = rc_dram[rb, 2 * hp:2 * hp + 2, :]
            src_rep = bass.AP(
                tensor=src.tensor, offset=src.offset,
                ap=[list(src.ap[0]), [0, 64], list(src.ap[1])])
            nc.gpsimd.dma_start(out=bc, in_=src_rep)
            nc.vector.tensor_tensor(
                out=o_tiles[hp], in0=o_tiles[hp], in1=bc, op=MUL)

    def yproj(rb, rc, o_tiles):
        ysb = p_y.tile([128, HID], F32, name="ysb", tag="y")
        for half in range(2):
            ps = ps_m.tile([128, 512], F32, name="psy", tag="m")
            for fc in range(FC):
                nc.tensor.matmul(
                    ps,
                    o_tiles[fc][:, rc * 128:(rc + 1) * 128],
                    wo_sb[fc][:, half * 512:(half + 1) * 512],
                    start=(fc == 0), stop=(fc == FC - 1))
            nc.vector.tensor_tensor(
                out=ysb[:, half * 512:(half + 1) * 512],
                in0=ps, in1=bias_sb[:, half * 512:(half + 1) * 512], op=ADD)
        row0 = rb * RB + rc * 128
        nc.sync.dma_start(out=y[row0:row0 + 128, :], in_=ysb)

    # ---- main pipeline ----
    qts_cur = qproj(0)
    o_prev = None
    qts_next = None
    for rb in range(NRB):
        o_cur = []
        qts_next = [] if rb + 1 < NRB else None
        ats_next = None
        for hp in range(FC):
            o_cur.append(attn(rb, hp, qts_cur))
            if qts_next is not None:
                if hp == 0:
                    ats_next = qload(rb + 1)
                qproj_fc(rb + 1, hp, ats_next, qts_next)
            if o_prev is not None and hp % 2 == 1:
                yproj(rb - 1, hp // 2, o_prev)
            if hp in (2, 4, 6):
                norm_half(rb, hp // 2 - 1, o_cur)
        norm_half(rb, 3, o_cur)
        o_prev, qts_cur = o_cur, qts_next
    for rc in range(4):
        yproj(NRB - 1, rc, o_prev)


def _build_nc():
    if "nc" in _CACHE:
        return _CACHE["nc"]
    from contextlib import ExitStack
    import concourse.tile as tile
    from concourse import bacc, mybir

    F32, BF16 = mybir.dt.float32, mybir.dt.bfloat16
    nc = bacc.Bacc("TRN2", target_bir_lowering=False, debug=False,
                   num_devices=NCORES)
    at = nc.dram_tensor("at", [HID, R], BF16, kind="ExternalInput").ap()
    ct = nc.dram_tensor("ct", [CTX, M], BF16, kind="ExternalInput").ap()
    wq = nc.dram_tensor("wq", [HID, HID], BF16, kind="ExternalInput").ap()
    wk = nc.dram_tensor("wk", [CTX, HID], BF16, kind="ExternalInput").ap()
    wv = nc.dram_tensor("wv", [CTX, HID], BF16, kind="ExternalInput").ap()
    wo = nc.dram_tensor("wo", [HID, HID], F32, kind="ExternalInput").ap()
    bo = nc.dram_tensor("bo", [1, HID], F32, kind="ExternalInput").ap()
    y = nc.dram_tensor("y", [R, HID], F32, kind="ExternalOutput").ap()

    with tile.TileContext(nc) as tc:
        with ExitStack() as ctx_stack:
            _body(tc, ctx_stack, at, ct, wq, wk, wv, wo, bo, y)
    nc.compile()
    _CACHE["nc"] = nc
    return nc


def _prep_in_maps(tokens, context, Wq, Wk, Wv, Wo, bo):
    bf16 = ml_dtypes.bfloat16
    tok_bf = tokens.astype(bf16)
    ctx_bf = context.astype(bf16)
    wq_bf = np.ascontiguousarray(Wq.astype(bf16))
    wk_bf = np.ascontiguousarray(Wk.astype(bf16))
    wv_bf = np.ascontiguousarray(Wv.astype(bf16))
    wo_f = np.ascontiguousarray(Wo.astype(np.float32))
    bo_f = np.ascontiguousarray(bo.reshape(1, HID).astype(np.float32))
    in_maps = []
    for c in range(NCORES):
        b, half = divmod(c, 2)
        at_np = np.ascontiguousarray(tok_bf[b, half * R:(half + 1) * R, :].T)
        ct_np = np.ascontiguousarray(ctx_bf[b].T)
        in_maps.append({
            "at": at_np, "ct": ct_np,
            "wq": wq_bf, "wk": wk_bf, "wv": wv_bf, "wo": wo_f,
            "bo": bo_f,
        })
    return in_maps


def kernel(tokens, context, Wq, Wk, Wv, Wo, bo):
    from concourse.bass_utils import run_bass_kernel_spmd

    tokens = np.asarray(tokens)
    context = np.asarray(context)
    Wq, Wk, Wv, Wo, bo = (np.asarray(a) for a in (Wq, Wk, Wv, Wo, bo))
    nc = _build_nc()
    in_maps = _prep_in_maps(tokens, context, Wq, Wk, Wv, Wo, bo)
    res = run_bass_kernel_spmd(nc, in_maps, core_ids=list(range(NCORES)))
    out = np.empty((B, N, HID), dtype=np.float32)
    for c in range(NCORES):
        b, half = divmod(c, 2)
        out[b, half * R:(half + 1) * R, :] = res.results[c]["y"]
    return out



# revision 7
# speedup vs baseline: 1.0574x; 1.0574x over previous
"""Multi-head cross-attention Trainium2 kernel (8 NeuronCores, SPMD).

Problem: tokens [4, 4096, 1024], context [4, 1024, 768], 16 heads, d=64.
  Q = tokens @ Wq; K = context @ Wk; V = context @ Wv
  out = softmax(Q K^T / 8) V  -> @ Wo + bo

Sharding: 8 cores = (batch b in 0..3) x (query-row half in 0..1).
Each core handles 2048 query rows of one batch against that batch's full
1024-key context.  No collectives; host concatenates the 8 output chunks.

Per-core dataflow (cost model charges matmuls by OUTPUT FREE SIZE only, so
the P@V stage is oriented out=[rows, d] (free 64/instr) instead of
[d, rows] (free 512) - this halves P@V PE time vs the naive orientation):
  - host passes tokens-chunk TRANSPOSED (at = [1024, 2048]) and context
    transposed (ct = [768, 1024]); all on-device layouts avoid transposes
    except O, which uses the XBAR DMA transpose (off the PE/PSUM path).
  - K^T proj:  kt[hp] = [128 feats (2 heads), 1024 keys] f32r
  - V proj:    vp[kc] = [128 keys, 16 heads x 64] bf16 (no ones column)
  - Q^T proj per row-block of 256 rows: qt[hp] = [128 feats, 256] f32r
  - attention per (row-block, head-QUAD g): S^T tiles [128 keys, 4x256 rows]
    via k=64 f32r matmuls; exp on ScalarE ([128,1024] per instr, scale=1/8);
    P@V per (head, rowchunk rc, kc): out psO[rc][128 rows, 64] accumulated
    over kc; denominators via rhs=ones [128,1] matmuls (free size 1 ~ free
    on PE) into psD[128, 16] columns.
  - per head-WAVE (8 heads): reciprocal of dens on DVE, normalization fused
    into the PSUM->SBUF copy via a stride-0-broadcast tensor_tensor, then
    XBAR DMA-transpose [rows,feat]->[feat,rows] chunks into oT tiles.
  - Y = oT^T-contract Wo (moving f32r) + bias, written out f32.
"""

import numpy as np
import ml_dtypes

B = 4
N = 4096
HID = 1024
CTX = 768
M = 1024          # context length (keys)
H = 16
D = 64
NCORES = 8
R = N * B // NCORES   # 2048 query rows per core
RB = 256              # row block
NRB = R // RB         # 8
FC = HID // 128       # 8 feature chunks == head pairs
KC = M // 128         # 8 key chunks
ICQ = HID // 128      # 8 contraction chunks for Q/Y proj
ICC = CTX // 128      # 6 contraction chunks for K/V proj
SCALE = D ** -0.5

_CACHE = {}


def _body(tc, ctx_stack, at, ct, wq, wk, wv, wo, bo, y):
    import concourse.bass as bass
    from concourse import mybir

    nc = tc.nc
    F32, BF16 = mybir.dt.float32, mybir.dt.bfloat16
    F32R = mybir.dt.float32r
    EXP = mybir.ActivationFunctionType.Exp
    MUL = mybir.AluOpType.mult
    ADD = mybir.AluOpType.add
    enter = ctx_stack.enter_context

    p_w = enter(tc.tile_pool(name="p_w", bufs=18))
    p_wq = enter(tc.tile_pool(name="p_wq", bufs=8))
    p_wo = enter(tc.tile_pool(name="p_wo", bufs=8))
    p_kt = enter(tc.tile_pool(name="p_kt", bufs=8))
    p_vp = enter(tc.tile_pool(name="p_vp", bufs=8))
    p_at = enter(tc.tile_pool(name="p_at", bufs=16))
    p_qt = enter(tc.tile_pool(name="p_qt", bufs=16))
    p_e = enter(tc.tile_pool(name="p_e", bufs=4))
    p_on = enter(tc.tile_pool(name="p_on", bufs=4))
    p_oT = enter(tc.tile_pool(name="p_oT", bufs=16))
    p_sm = enter(tc.tile_pool(name="p_sm", bufs=4))
    p_y = enter(tc.tile_pool(name="p_y", bufs=2))
    p_1 = enter(tc.tile_pool(name="p_1", bufs=2))
    ps_s = enter(tc.tile_pool(name="ps_s", bufs=2, space="PSUM"))
    ps_o = enter(tc.tile_pool(name="ps_o", bufs=2, space="PSUM"))
    ps_d = enter(tc.tile_pool(name="ps_d", bufs=1, space="PSUM"))
    ps_m = enter(tc.tile_pool(name="ps_m", bufs=1, space="PSUM"))

    # ---- PE warm-up: ~3.5us of dummy matmuls during the initial DMA
    # window flips the HAM clock gate to 2.4 GHz before real work ----
    warm_t = p_y.tile([128, HID], F32, name="warm_t", tag="y")
    nc.vector.memset(warm_t[:, 0:512], 0.0)
    for i in range(3):
        wps = ps_m.tile([128, 512], F32, name="wps", tag="m")
        nc.tensor.matmul(wps, warm_t[:, 0:128], warm_t[:, 0:512],
                         start=True, stop=True)
    # dummy exp pre-loads the ScalarE activation table set off the
    # attention critical path
    nc.scalar.activation(warm_t[:, 8:16], warm_t[:, 0:8], EXP, scale=SCALE)

    # ---- bias broadcast [1, HID] -> [128, HID] (gpsimd 0-step DMA) ----
    bias_sb = p_1.tile([128, HID], F32, name="bias_sb", tag="bias")
    nc.gpsimd.dma_start(
        out=bias_sb,
        in_=bass.AP(tensor=bo.tensor, offset=bo.offset,
                    ap=[[0, 128]] + [list(a) for a in bo.ap[1:]]),
    )
    ones_bf = p_1.tile([128, 1], BF16, name="ones_bf", tag="one")
    nc.vector.memset(ones_bf, 1.0)

    # ---- load ct / wk / wv (all resident; tag "w") ----
    ct_sb = []
    for i in range(ICC):
        t = p_w.tile([128, M], BF16, name=f"ct{i}", tag="w")
        nc.sync.dma_start(out=t, in_=ct[i * 128:(i + 1) * 128, :])
        ct_sb.append(t)
    wk_sb = []
    for i in range(ICC):
        t = p_w.tile([128, HID], BF16, name=f"wk{i}", tag="w")
        nc.sync.dma_start(out=t, in_=wk[i * 128:(i + 1) * 128, :])
        wk_sb.append(t)
    wv_sb = []
    for i in range(ICC):
        t = p_w.tile([128, HID], BF16, name=f"wv{i}", tag="w")
        nc.sync.dma_start(out=t, in_=wv[i * 128:(i + 1) * 128, :])
        wv_sb.append(t)
    wq_sb = []
    for i in range(ICQ):
        t = p_wq.tile([128, HID], BF16, name=f"wq{i}", tag="wq")
        nc.sync.dma_start(out=t, in_=wq[i * 128:(i + 1) * 128, :])
        wq_sb.append(t)
    wo_sb = []
    for i in range(FC):
        t = p_wo.tile([128, HID], BF16, name=f"wo{i}", tag="wo")
        nc.sync.dma_start(out=t, in_=wo[i * 128:(i + 1) * 128, :])
        wo_sb.append(t)

    # ---- K^T projection: kt[hp] = [128 feats (2 heads), M keys] f32r ----
    kt_sb = []
    for fc in range(FC):
        kt = p_kt.tile([128, M], F32R, name=f"kt{fc}", tag="kt")
        for half in range(2):
            ps = ps_m.tile([128, 512], F32, name="psk", tag="m")
            for ic in range(ICC):
                nc.tensor.matmul(
                    ps,
                    wk_sb[ic][:, fc * 128:(fc + 1) * 128],
                    ct_sb[ic][:, half * 512:(half + 1) * 512],
                    start=(ic == 0), stop=(ic == ICC - 1),
                )
            nc.vector.tensor_copy(kt[:, half * 512:(half + 1) * 512], ps)
        kt_sb.append(kt)

    # ---- V projection into [keys 128, 16 heads x 64] bf16 ----
    vp_sb = []
    for kc in range(KC):
        vp = p_vp.tile([128, H * D], BF16, name=f"vp{kc}", tag="vp")
        for half in range(2):
            ps = ps_m.tile([128, 512], F32, name="psv", tag="m")
            for ic in range(ICC):
                nc.tensor.matmul(
                    ps,
                    ct_sb[ic][:, kc * 128:(kc + 1) * 128],
                    wv_sb[ic][:, half * 512:(half + 1) * 512],
                    start=(ic == 0), stop=(ic == ICC - 1),
                )
            nc.vector.tensor_copy(vp[:, half * 512:(half + 1) * 512], ps)
        vp_sb.append(vp)

    def qload(rb):
        ats = []
        for ic in range(ICQ):
            a = p_at.tile([128, RB], BF16, name=f"at{ic}_{rb}", tag="at")
            nc.sync.dma_start(
                out=a, in_=at[ic * 128:(ic + 1) * 128, rb * RB:(rb + 1) * RB])
            ats.append(a)
        return ats

    def qproj_fc(rb, fc, ats, qts):
        ps = ps_m.tile([128, 512], F32, name="psq", tag="m")
        for ic in range(ICQ):
            nc.tensor.matmul(
                ps[:, 0:RB], wq_sb[ic][:, fc * 128:(fc + 1) * 128], ats[ic],
                start=(ic == 0), stop=(ic == ICQ - 1))
        qt = p_qt.tile([128, RB], F32R, name=f"qt{fc}_{rb}", tag="qt")
        nc.vector.tensor_copy(qt, ps[:, 0:RB])
        qts.append(qt)

    def attn_quad_kc(rb, g, kc, qts, psO, psD):
        """Heads 4g..4g+3 for one key chunk: S^T, exp, P@V, denominators.

        PSUM start=True zeroes the ENTIRE 2KB bank, so each bank gets
        exactly one start: quarters 0/2 of sQ start their banks (quarters
        1/3 accumulate into the pending-zeroed remainder), and the psO/psD
        banks are started only by the very first matmul of each wave.
        """
        # Column position of head j in sQ/e: heads contracting partitions
        # 0:64 (j even) fill bank A (cols 0:512), heads on 64:128 (j odd)
        # fill bank B — matmuls sharing a psum bank MUST share the same
        # contraction partition range (runtime faults otherwise).
        pos = lambda j: (j % 2) * 2 + j // 2
        sQ = ps_s.tile([128, 4 * RB], F32, name="sQ", tag="s")
        with tc.high_priority(offset=100):
            for j in range(4):
                h = 4 * g + j
                hp, i = divmod(h, 2)
                nc.tensor.matmul(
                    sQ[:, pos(j) * RB:(pos(j) + 1) * RB],
                    kt_sb[hp][64 * i:64 * (i + 1), kc * 128:(kc + 1) * 128],
                    qts[hp][64 * i:64 * (i + 1), :],
                    start=(j // 2 == 0), stop=(j // 2 == 1),
                    skip_group_check=True)
        e = p_e.tile([128, 4 * RB], BF16, name="e", tag="e")
        nc.scalar.activation(e, sQ, EXP, scale=SCALE)
        first = (g % 2 == 0) and (kc == 0)
        last = (g % 2 == 1) and (kc == KC - 1)
        for j in range(4):
            h = 4 * g + j
            hw_ = 4 * (g % 2) + j      # head index within the 8-head wave
            for rc in range(2):
                el = e[:, pos(j) * RB + rc * 128: pos(j) * RB + (rc + 1) * 128]
                nc.tensor.matmul(
                    psO[rc][:, hw_ * 64:(hw_ + 1) * 64],
                    el, vp_sb[kc][:, h * 64:(h + 1) * 64],
                    start=(first and j == 0), stop=(last and j == 3),
                    skip_group_check=True)
                c = hw_ * 2 + rc
                nc.tensor.matmul(
                    psD[:, c:c + 1], el, ones_bf,
                    start=(first and j == 0 and rc == 0),
                    stop=(last and j == 3 and rc == 1),
                    skip_group_check=True)

    def wave_drain(rb, w, psO, psD, oT_list):
        """Normalize the 8 finished heads and DMA-transpose into oT tiles."""
        dcp = p_sm.tile([128, 16], F32, name="dcp", tag="sm")
        nc.vector.tensor_copy(dcp, psD)
        rcp = p_sm.tile([128, 16], F32, name="rcp", tag="sm")
        nc.vector.reciprocal_approx_fast(out=rcp, in_=dcp)
        for rc in range(2):
            on = p_on.tile([128, 512], BF16, name=f"on{w}_{rc}", tag="on")
            src = rcp[:, rc:rc + 1]
            rep = bass.AP(tensor=src.tensor, offset=src.offset,
                          ap=[list(src.ap[0]), [2, 8], [0, 64]])
            nc.vector.tensor_tensor(out=on, in0=psO[rc], in1=rep, op=MUL)
            for c2 in range(4):        # head-pair chunks within the wave
                hp = 4 * w + c2
                nc.sync.dma_start(
                    out=oT_list[hp][:, rc * 128:(rc + 1) * 128],
                    in_=on[:, c2 * 128:(c2 + 1) * 128], transpose=True)

    def yproj_piece(rb, piece, oT_list, ysb_box):
        rc, half = divmod(piece, 2)
        if half == 0:
            ysb_box[rc] = p_y.tile([128, HID], F32, name=f"ysb{rc}", tag="y")
        ps = ps_m.tile([128, 512], F32, name="psy", tag="m")
        for fc in range(FC):
            nc.tensor.matmul(
                ps, oT_list[fc][:, rc * 128:(rc + 1) * 128],
                wo_sb[fc][:, half * 512:(half + 1) * 512],
                start=(fc == 0), stop=(fc == FC - 1))
        ysb = ysb_box[rc]
        nc.vector.tensor_tensor(
            out=ysb[:, half * 512:(half + 1) * 512],
            in0=ps, in1=bias_sb[:, half * 512:(half + 1) * 512], op=ADD)
        if half == 1:
            row0 = rb * RB + rc * 128
            nc.sync.dma_start(out=y[row0:row0 + 128, :], in_=ysb)

    # ---- main pipeline ----
    ats0 = qload(0)
    qts_cur = []
    for fc in range(FC):
        qproj_fc(0, fc, ats0, qts_cur)

    oT_prev = None
    for rb in range(NRB):
        oT_cur = [p_oT.tile([128, RB], BF16, name=f"oT{fc}_{rb}", tag="oT")
                  for fc in range(FC)]
        qts_next = [] if rb + 1 < NRB else None
        ats_next = None
        ysb_box = [None, None]
        psO = None
        for g in range(4):
            if g % 2 == 0:
                psO = [ps_o.tile([128, 512], F32, name=f"psO{rc}", tag="o")
                       for rc in range(2)]
                psD = ps_d.tile([128, 16], F32, name="psD", tag="d")
            for kc in range(KC):
                attn_quad_kc(rb, g, kc, qts_cur, psO, psD)
                if qts_next is not None:
                    if g == 0 and kc == 0:
                        ats_next = qload(rb + 1)
                    if kc == 2:
                        qproj_fc(rb + 1, 2 * g, ats_next, qts_next)
                    elif kc == 5:
                        qproj_fc(rb + 1, 2 * g + 1, ats_next, qts_next)
            if oT_prev is not None:
                yproj_piece(rb - 1, g, oT_prev, ysb_box)
            if g % 2 == 1:
                wave_drain(rb, g // 2, psO, psD, oT_cur)
        oT_prev, qts_cur = oT_cur, qts_next
    ysb_box = [None, None]
    for piece in range(4):
        yproj_piece(NRB - 1, piece, oT_prev, ysb_box)


def _build_nc():
    if "nc" in _CACHE:
        return _CACHE["nc"]
    from contextlib import ExitStack
    import concourse.tile as tile
    from concourse import bacc, mybir

    F32, BF16 = mybir.dt.float32, mybir.dt.bfloat16
    nc = bacc.Bacc("TRN2", target_bir_lowering=False, debug=False,
                   num_devices=NCORES)
    at = nc.dram_tensor("at", [HID, R], BF16, kind="ExternalInput").ap()
    ct = nc.dram_tensor("ct", [CTX, M], BF16, kind="ExternalInput").ap()
    wq = nc.dram_tensor("wq", [HID, HID], BF16, kind="ExternalInput").ap()
    wk = nc.dram_tensor("wk", [CTX, HID], BF16, kind="ExternalInput").ap()
    wv = nc.dram_tensor("wv", [CTX, HID], BF16, kind="ExternalInput").ap()
    wo = nc.dram_tensor("wo", [HID, HID], BF16, kind="ExternalInput").ap()
    bo = nc.dram_tensor("bo", [1, HID], F32, kind="ExternalInput").ap()
    y = nc.dram_tensor("y", [R, HID], F32, kind="ExternalOutput").ap()

    with tile.TileContext(nc) as tc:
        with ExitStack() as ctx_stack:
            _body(tc, ctx_stack, at, ct, wq, wk, wv, wo, bo, y)
    nc.compile()
    _CACHE["nc"] = nc
    return nc


def _prep_in_maps(tokens, context, Wq, Wk, Wv, Wo, bo):
    bf16 = ml_dtypes.bfloat16
    tok_bf = tokens.astype(bf16)
    ctx_bf = context.astype(bf16)
    wq_bf = np.ascontiguousarray(Wq.astype(bf16))
    wk_bf = np.ascontiguousarray(Wk.astype(bf16))
    wv_bf = np.ascontiguousarray(Wv.astype(bf16))
    wo_f = np.ascontiguousarray(Wo.astype(bf16))
    bo_f = np.ascontiguousarray(bo.reshape(1, HID).astype(np.float32))
    in_maps = []
    for c in range(NCORES):
        b, half = divmod(c, 2)
        at_np = np.ascontiguousarray(tok_bf[b, half * R:(half + 1) * R, :].T)
        ct_np = np.ascontiguousarray(ctx_bf[b].T)
        in_maps.append({
            "at": at_np, "ct": ct_np,
            "wq": wq_bf, "wk": wk_bf, "wv": wv_bf, "wo": wo_f,
            "bo": bo_f,
        })
    return in_maps


def kernel(tokens, context, Wq, Wk, Wv, Wo, bo):
    from concourse.bass_utils import run_bass_kernel_spmd

    tokens = np.asarray(tokens)
    context = np.asarray(context)
    Wq, Wk, Wv, Wo, bo = (np.asarray(a) for a in (Wq, Wk, Wv, Wo, bo))
    nc = _build_nc()
    in_maps = _prep_in_maps(tokens, context, Wq, Wk, Wv, Wo, bo)
    res = run_bass_kernel_spmd(nc, in_maps, core_ids=list(range(NCORES)))
    out = np.empty((B, N, HID), dtype=np.float32)
    for c in range(NCORES):
        b, half = divmod(c, 2)
        out[b, half * R:(half + 1) * R, :] = res.results[c]["y"]
    return out


# revision 10
# speedup vs baseline: 1.1482x; 1.0859x over previous
"""Multi-head cross-attention Trainium2 kernel (8 NeuronCores, SPMD).

Problem: tokens [4, 4096, 1024], context [4, 1024, 768], 16 heads, d=64.
  Q = tokens @ Wq; K = context @ Wk; V = context @ Wv
  out = softmax(Q K^T / 8) V  -> @ Wo + bo

Sharding: 8 cores = (batch b in 0..3) x (query-row half in 0..1).
Each core handles 2048 query rows of one batch against that batch's full
1024-key context.  No collectives; host concatenates the 8 output chunks.

Per-core dataflow (cost model charges matmuls by OUTPUT FREE SIZE only, so
the P@V stage is oriented out=[rows, d] (free 64/instr) instead of
[d, rows] (free 512) - this halves P@V PE time vs the naive orientation):
  - host passes tokens-chunk TRANSPOSED (at = [1024, 2048]) and context
    transposed (ct = [768, 1024]); all on-device layouts avoid transposes
    except O, which uses the XBAR DMA transpose (off the PE/PSUM path).
  - K^T proj:  kt[hp] = [128 feats (2 heads), 1024 keys] f32r
  - V proj:    vp[kc] = [128 keys, 16 heads x 64] bf16 (no ones column)
  - Q^T proj per row-block of 256 rows: qt[hp] = [128 feats, 256] f32r
  - attention per (row-block, head-QUAD g): S^T tiles [128 keys, 4x256 rows]
    via k=64 f32r matmuls; exp on ScalarE ([128,1024] per instr, scale=1/8);
    P@V per (head, rowchunk rc, kc): out psO[rc][128 rows, 64] accumulated
    over kc; denominators via rhs=ones [128,1] matmuls (free size 1 ~ free
    on PE) into psD[128, 16] columns.
  - per head-WAVE (8 heads): reciprocal of dens on DVE, normalization fused
    into the PSUM->SBUF copy via a stride-0-broadcast tensor_tensor, then
    XBAR DMA-transpose [rows,feat]->[feat,rows] chunks into oT tiles.
  - Y = oT^T-contract Wo (moving f32r) + bias, written out f32.
"""

import numpy as np
import ml_dtypes

B = 4
N = 4096
HID = 1024
CTX = 768
M = 1024          # context length (keys)
H = 16
D = 64
NCORES = 8
R = N * B // NCORES   # 2048 query rows per core
RB = 256              # row block
NRB = R // RB         # 8
FC = HID // 128       # 8 feature chunks == head pairs
KC = M // 128         # 8 key chunks
ICQ = HID // 128      # 8 contraction chunks for Q/Y proj
ICC = CTX // 128      # 6 contraction chunks for K/V proj
SCALE = D ** -0.5

_CACHE = {}


def _body(tc, ctx_stack, at, ct, wq, wk, wv, wo, bo, y):
    import concourse.bass as bass
    from concourse import mybir

    nc = tc.nc
    F32, BF16 = mybir.dt.float32, mybir.dt.bfloat16
    F32R = mybir.dt.float32r
    EXP = mybir.ActivationFunctionType.Exp
    MUL = mybir.AluOpType.mult
    ADD = mybir.AluOpType.add
    enter = ctx_stack.enter_context

    p_w = enter(tc.tile_pool(name="p_w", bufs=18))
    p_wq = enter(tc.tile_pool(name="p_wq", bufs=8))
    p_wo = enter(tc.tile_pool(name="p_wo", bufs=8))
    p_kt = enter(tc.tile_pool(name="p_kt", bufs=8))
    p_vp = enter(tc.tile_pool(name="p_vp", bufs=8))
    p_at = enter(tc.tile_pool(name="p_at", bufs=16))
    p_qt = enter(tc.tile_pool(name="p_qt", bufs=16))
    p_e = enter(tc.tile_pool(name="p_e", bufs=4))
    p_on = enter(tc.tile_pool(name="p_on", bufs=4))
    p_oT = enter(tc.tile_pool(name="p_oT", bufs=16))
    p_sm = enter(tc.tile_pool(name="p_sm", bufs=4))
    p_y = enter(tc.tile_pool(name="p_y", bufs=2))
    p_1 = enter(tc.tile_pool(name="p_1", bufs=2))
    ps_s = enter(tc.tile_pool(name="ps_s", bufs=2, space="PSUM"))
    ps_o = enter(tc.tile_pool(name="ps_o", bufs=2, space="PSUM"))
    ps_d = enter(tc.tile_pool(name="ps_d", bufs=1, space="PSUM"))
    ps_m = enter(tc.tile_pool(name="ps_m", bufs=1, space="PSUM"))

    # ---- PE warm-up: ~3.5us of dummy matmuls during the initial DMA
    # window flips the HAM clock gate to 2.4 GHz before real work ----
    warm_t = p_y.tile([128, HID], F32, name="warm_t", tag="y")
    nc.vector.memset(warm_t[:, 0:512], 0.0)
    for i in range(3):
        wps = ps_m.tile([128, 512], F32, name="wps", tag="m")
        nc.tensor.matmul(wps, warm_t[:, 0:128], warm_t[:, 0:512],
                         start=True, stop=True)
    # dummy exp pre-loads the ScalarE activation table set off the
    # attention critical path
    nc.scalar.activation(warm_t[:, 8:16], warm_t[:, 0:8], EXP, scale=SCALE)

    # ---- bias broadcast [1, HID] -> [128, HID] (gpsimd 0-step DMA) ----
    bias_sb = p_1.tile([128, HID], F32, name="bias_sb", tag="bias")
    nc.gpsimd.dma_start(
        out=bias_sb,
        in_=bass.AP(tensor=bo.tensor, offset=bo.offset,
                    ap=[[0, 128]] + [list(a) for a in bo.ap[1:]]),
    )
    ones_bf = p_1.tile([128, 1], BF16, name="ones_bf", tag="one")
    nc.vector.memset(ones_bf, 1.0)

    # ---- load ct / wk / wv (all resident; tag "w") ----
    ct_sb = []
    for i in range(ICC):
        t = p_w.tile([128, M], BF16, name=f"ct{i}", tag="w")
        nc.sync.dma_start(out=t, in_=ct[i * 128:(i + 1) * 128, :])
        ct_sb.append(t)
    wk_sb = []
    for i in range(ICC):
        t = p_w.tile([128, HID], BF16, name=f"wk{i}", tag="w")
        nc.sync.dma_start(out=t, in_=wk[i * 128:(i + 1) * 128, :])
        wk_sb.append(t)
    wv_sb = []
    for i in range(ICC):
        t = p_w.tile([128, HID], BF16, name=f"wv{i}", tag="w")
        nc.sync.dma_start(out=t, in_=wv[i * 128:(i + 1) * 128, :])
        wv_sb.append(t)
    wq_sb = []
    for i in range(ICQ):
        t = p_wq.tile([128, HID], BF16, name=f"wq{i}", tag="wq")
        nc.sync.dma_start(out=t, in_=wq[i * 128:(i + 1) * 128, :])
        wq_sb.append(t)
    wo_sb = []
    for i in range(FC):
        t = p_wo.tile([128, HID], BF16, name=f"wo{i}", tag="wo")
        nc.sync.dma_start(out=t, in_=wo[i * 128:(i + 1) * 128, :])
        wo_sb.append(t)

    # ---- startup projections (K, V, Q-rb0) fan out across the idle
    # attention psum banks: 2 ps_s tiles give 4 bank-targets, 2 ps_o tiles
    # and the ps_m slot give 3 more -> 7-deep fill pipeline instead of
    # serializing every fill through the single ps_m bank ----
    startup_tgts = []
    for si in range(2):
        t = ps_s.tile([128, 4 * RB], F32, name=f"su_s{si}", tag="s")
        startup_tgts.append(t[:, 0:512])
        startup_tgts.append(t[:, 512:1024])
    for oi in range(2):
        t = ps_o.tile([128, 512], F32, name=f"su_o{oi}", tag="o")
        startup_tgts.append(t)
    startup_tgts.append(ps_m.tile([128, 512], F32, name="su_m", tag="m"))
    su_idx = [0]

    def sfill(width, emit, out_sb, cast=None):
        tgt = startup_tgts[su_idx[0] % len(startup_tgts)]
        su_idx[0] += 1
        emit(tgt[:, 0:width])
        nc.vector.tensor_copy(out_sb, tgt[:, 0:width])

    # ---- K^T projection: kt[hp] = [128 feats (2 heads), M keys] f32r ----
    kt_sb = []
    for fc in range(FC):
        kt = p_kt.tile([128, M], F32R, name=f"kt{fc}", tag="kt")
        for half in range(2):
            def emit_k(tgt, fc=fc, half=half):
                for ic in range(ICC):
                    nc.tensor.matmul(
                        tgt,
                        wk_sb[ic][:, fc * 128:(fc + 1) * 128],
                        ct_sb[ic][:, half * 512:(half + 1) * 512],
                        start=(ic == 0), stop=(ic == ICC - 1),
                        skip_group_check=True,
                    )
            sfill(512, emit_k, kt[:, half * 512:(half + 1) * 512])
        kt_sb.append(kt)

    # ---- V projection into [keys 128, 16 heads x 64] bf16 ----
    vp_sb = []
    for kc in range(KC):
        vp = p_vp.tile([128, H * D], BF16, name=f"vp{kc}", tag="vp")
        for half in range(2):
            def emit_v(tgt, kc=kc, half=half):
                for ic in range(ICC):
                    nc.tensor.matmul(
                        tgt,
                        ct_sb[ic][:, kc * 128:(kc + 1) * 128],
                        wv_sb[ic][:, half * 512:(half + 1) * 512],
                        start=(ic == 0), stop=(ic == ICC - 1),
                        skip_group_check=True,
                    )
            sfill(512, emit_v, vp[:, half * 512:(half + 1) * 512])
        vp_sb.append(vp)

    def qload(rb):
        ats = []
        for ic in range(ICQ):
            a = p_at.tile([128, RB], BF16, name=f"at{ic}_{rb}", tag="at")
            nc.sync.dma_start(
                out=a, in_=at[ic * 128:(ic + 1) * 128, rb * RB:(rb + 1) * RB])
            ats.append(a)
        return ats

    def qproj_fc(rb, fc, ats, qts, startup=False):
        qt = p_qt.tile([128, RB], F32R, name=f"qt{fc}_{rb}", tag="qt")

        def emit_q(tgt):
            for ic in range(ICQ):
                nc.tensor.matmul(
                    tgt, wq_sb[ic][:, fc * 128:(fc + 1) * 128], ats[ic],
                    start=(ic == 0), stop=(ic == ICQ - 1),
                    skip_group_check=True)

        if startup:
            sfill(RB, emit_q, qt)
        else:
            ps = ps_m.tile([128, 512], F32, name="psq", tag="m")
            emit_q(ps[:, 0:RB])
            nc.vector.tensor_copy(qt, ps[:, 0:RB])
        qts.append(qt)

    def attn_quad_kc(rb, g, kc, qts, psO, psD):
        """Heads 4g..4g+3 for one key chunk: S^T, exp, P@V, denominators.

        PSUM start=True zeroes the ENTIRE 2KB bank, so each bank gets
        exactly one start: quarters 0/2 of sQ start their banks (quarters
        1/3 accumulate into the pending-zeroed remainder), and the psO/psD
        banks are started only by the very first matmul of each wave.
        """
        # Column position of head j in sQ/e: heads contracting partitions
        # 0:64 (j even) fill bank A (cols 0:512), heads on 64:128 (j odd)
        # fill bank B — matmuls sharing a psum bank MUST share the same
        # contraction partition range (runtime faults otherwise).
        pos = lambda j: (j % 2) * 2 + j // 2
        sQ = ps_s.tile([128, 4 * RB], F32, name="sQ", tag="s")
        with tc.high_priority(offset=100):
            for j in range(4):
                h = 4 * g + j
                hp, i = divmod(h, 2)
                nc.tensor.matmul(
                    sQ[:, pos(j) * RB:(pos(j) + 1) * RB],
                    kt_sb[hp][64 * i:64 * (i + 1), kc * 128:(kc + 1) * 128],
                    qts[hp][64 * i:64 * (i + 1), :],
                    start=(j // 2 == 0), stop=(j // 2 == 1),
                    skip_group_check=True)
        e = p_e.tile([128, 4 * RB], BF16, name="e", tag="e")
        nc.scalar.activation(e, sQ, EXP, scale=SCALE)
        first = (g % 2 == 0) and (kc == 0)
        last = (g % 2 == 1) and (kc == KC - 1)
        for j in range(4):
            h = 4 * g + j
            hw_ = 4 * (g % 2) + j      # head index within the 8-head wave
            for rc in range(2):
                el = e[:, pos(j) * RB + rc * 128: pos(j) * RB + (rc + 1) * 128]
                nc.tensor.matmul(
                    psO[rc][:, hw_ * 64:(hw_ + 1) * 64],
                    el, vp_sb[kc][:, h * 64:(h + 1) * 64],
                    start=(first and j == 0), stop=(last and j == 3),
                    skip_group_check=True)
                c = hw_ * 2 + rc
                nc.tensor.matmul(
                    psD[:, c:c + 1], el, ones_bf,
                    start=(first and j == 0 and rc == 0),
                    stop=(last and j == 3 and rc == 1),
                    skip_group_check=True)

    def wave_drain(rb, w, psO, psD, oT_list):
        """Normalize the 8 finished heads and DMA-transpose into oT tiles."""
        dcp = p_sm.tile([128, 16], F32, name="dcp", tag="sm")
        nc.vector.tensor_copy(dcp, psD)
        rcp = p_sm.tile([128, 16], F32, name="rcp", tag="sm")
        nc.vector.reciprocal_approx_fast(out=rcp, in_=dcp)
        for rc in range(2):
            on = p_on.tile([128, 512], BF16, name=f"on{w}_{rc}", tag="on")
            src = rcp[:, rc:rc + 1]
            rep = bass.AP(tensor=src.tensor, offset=src.offset,
                          ap=[list(src.ap[0]), [2, 8], [0, 64]])
            nc.vector.tensor_tensor(out=on, in0=psO[rc], in1=rep, op=MUL)
            for c2 in range(4):        # head-pair chunks within the wave
                hp = 4 * w + c2
                nc.sync.dma_start(
                    out=oT_list[hp][:, rc * 128:(rc + 1) * 128],
                    in_=on[:, c2 * 128:(c2 + 1) * 128], transpose=True)

    def yproj_piece(rb, piece, oT_list, ysb_box):
        rc, half = divmod(piece, 2)
        if half == 0:
            ysb_box[rc] = p_y.tile([128, HID], F32, name=f"ysb{rc}", tag="y")
        ps = ps_m.tile([128, 512], F32, name="psy", tag="m")
        for fc in range(FC):
            nc.tensor.matmul(
                ps, oT_list[fc][:, rc * 128:(rc + 1) * 128],
                wo_sb[fc][:, half * 512:(half + 1) * 512],
                start=(fc == 0), stop=(fc == FC - 1))
        ysb = ysb_box[rc]
        nc.vector.tensor_tensor(
            out=ysb[:, half * 512:(half + 1) * 512],
            in0=ps, in1=bias_sb[:, half * 512:(half + 1) * 512], op=ADD)
        if half == 1:
            row0 = rb * RB + rc * 128
            nc.sync.dma_start(out=y[row0:row0 + 128, :], in_=ysb)

    # ---- main pipeline ----
    ats0 = qload(0)
    qts_cur = []
    for fc in range(FC):
        qproj_fc(0, fc, ats0, qts_cur, startup=True)

    oT_prev = None
    for rb in range(NRB):
        oT_cur = [p_oT.tile([128, RB], BF16, name=f"oT{fc}_{rb}", tag="oT")
                  for fc in range(FC)]
        qts_next = [] if rb + 1 < NRB else None
        ats_next = None
        ysb_box = [None, None]
        psO = None
        for g in range(4):
            if g % 2 == 0:
                psO = [ps_o.tile([128, 512], F32, name=f"psO{rc}", tag="o")
                       for rc in range(2)]
                psD = ps_d.tile([128, 16], F32, name="psD", tag="d")
            for kc in range(KC):
                attn_quad_kc(rb, g, kc, qts_cur, psO, psD)
                if qts_next is not None:
                    if g == 0 and kc == 0:
                        ats_next = qload(rb + 1)
                    if kc == 2:
                        qproj_fc(rb + 1, 2 * g, ats_next, qts_next)
                    elif kc == 5:
                        qproj_fc(rb + 1, 2 * g + 1, ats_next, qts_next)
            if oT_prev is not None:
                yproj_piece(rb - 1, g, oT_prev, ysb_box)
            if g % 2 == 1:
                wave_drain(rb, g // 2, psO, psD, oT_cur)
        oT_prev, qts_cur = oT_cur, qts_next
    ysb_box = [None, None]
    for piece in range(4):
        yproj_piece(NRB - 1, piece, oT_prev, ysb_box)


def _build_nc():
    if "nc" in _CACHE:
        return _CACHE["nc"]
    from contextlib import ExitStack
    import concourse.tile as tile
    from concourse import bacc, mybir

    F32, BF16 = mybir.dt.float32, mybir.dt.bfloat16
    nc = bacc.Bacc("TRN2", target_bir_lowering=False, debug=False,
                   num_devices=NCORES)
    at = nc.dram_tensor("at", [HID, R], BF16, kind="ExternalInput").ap()
    ct = nc.dram_tensor("ct", [CTX, M], BF16, kind="ExternalInput").ap()
    wq = nc.dram_tensor("wq", [HID, HID], BF16, kind="ExternalInput").ap()
    wk = nc.dram_tensor("wk", [CTX, HID], BF16, kind="ExternalInput").ap()
    wv = nc.dram_tensor("wv", [CTX, HID], BF16, kind="ExternalInput").ap()
    wo = nc.dram_tensor("wo", [HID, HID], BF16, kind="ExternalInput").ap()
    bo = nc.dram_tensor("bo", [1, HID], F32, kind="ExternalInput").ap()
    y = nc.dram_tensor("y", [R, HID], F32, kind="ExternalOutput").ap()

    with tile.TileContext(nc) as tc:
        with ExitStack() as ctx_stack:
            _body(tc, ctx_stack, at, ct, wq, wk, wv, wo, bo, y)
    nc.compile()
    _CACHE["nc"] = nc
    return nc


def _prep_in_maps(tokens, context, Wq, Wk, Wv, Wo, bo):
    bf16 = ml_dtypes.bfloat16
    tok_bf = tokens.astype(bf16)
    ctx_bf = context.astype(bf16)
    wq_bf = np.ascontiguousarray(Wq.astype(bf16))
    wk_bf = np.ascontiguousarray(Wk.astype(bf16))
    wv_bf = np.ascontiguousarray(Wv.astype(bf16))
    wo_f = np.ascontiguousarray(Wo.astype(bf16))
    bo_f = np.ascontiguousarray(bo.reshape(1, HID).astype(np.float32))
    in_maps = []
    for c in range(NCORES):
        b, half = divmod(c, 2)
        at_np = np.ascontiguousarray(tok_bf[b, half * R:(half + 1) * R, :].T)
        ct_np = np.ascontiguousarray(ctx_bf[b].T)
        in_maps.append({
            "at": at_np, "ct": ct_np,
            "wq": wq_bf, "wk": wk_bf, "wv": wv_bf, "wo": wo_f,
            "bo": bo_f,
        })
    return in_maps


def kernel(tokens, context, Wq, Wk, Wv, Wo, bo):
    from concourse.bass_utils import run_bass_kernel_spmd

    tokens = np.asarray(tokens)
    context = np.asarray(context)
    Wq, Wk, Wv, Wo, bo = (np.asarray(a) for a in (Wq, Wk, Wv, Wo, bo))
    nc = _build_nc()
    in_maps = _prep_in_maps(tokens, context, Wq, Wk, Wv, Wo, bo)
    res = run_bass_kernel_spmd(nc, in_maps, core_ids=list(range(NCORES)))
    out = np.empty((B, N, HID), dtype=np.float32)
    for c in range(NCORES):
        b, half = divmod(c, 2)
        out[b, half * R:(half + 1) * R, :] = res.results[c]["y"]
    return out
